# revision 1
# baseline (speedup 1.0000x reference)
"""Trainium2 Bass kernel for EnhanceLayerLinear.

Computes out = GroupedLinear(Linear(x)):
    y = x @ W.T + b                      [B,S,D]
    out[..., g, :] = y[..., g, :] @ Wg[g].T + bg[g]   (block-diagonal, G groups)

Sharding: data-parallel over tokens (B*S = 8192 -> 1024 per core). Each core
runs both GEMM stages locally; the grouped stage shards trivially since it is
applied per token.

Stage 1 runs in bf16 (fp32 accumulate in psum): fp32r matmuls are limited to
~272 ns/MM by the 2-pass fp32 LDWEIGHTS (224 ns) that cannot hide behind a
213 ns matmul, and the hardware forbids mixing bf16 weights with fp32r
activations. Stage 2 (the small grouped matmul) runs in float32r - fp32
truncated to FP22 - off the psum evacuation, so y is never quantized to bf16;
each grouped matmul costs a ~422 ns PE slot (its 2-pass fp32 LDWEIGHTS
cannot be hidden) - 64 slots, ~13 us, the price of keeping y at FP22.

Layout trick: stage 1 computes y TRANSPOSED (features on partitions, tokens on
the free axis). That makes each 128-row psum tile exactly one group's slice
with the contraction axis of stage 2 already on partitions, so the grouped
matmul chains directly with zero on-chip transposes. The host hands the kernel
pre-transposed views of x / W / Wg and re-transposes the output.
"""

from collections import deque

import ml_dtypes
import numpy as np

import concourse.bacc as bacc
import concourse.bass as bass
import concourse.tile as tile
from concourse import mybir
from concourse import bass_utils

f32 = mybir.dt.float32
f32r = mybir.dt.float32r
bf16 = mybir.dt.bfloat16
ACT_ID = mybir.ActivationFunctionType.Identity

B, S, D = 4, 2048, 4096
T = B * S                 # 8192 tokens
G, IG = 32, 128           # groups x group size (4096 = 32*128)
NCORES = 8
TPC = T // NCORES         # 1024 tokens per core
KT = D // 128             # 32 contraction tiles
NMOV = 512                # moving free dim per matmul (= one psum bank of fp32)
NCH = TPC // NMOV         # 2 token chunks per core

_CACHE = {}


def _build():
    nc = bacc.Bacc("TRN2", target_bir_lowering=False, debug=False)
    # x_d[kt, tch, p, t] = x[core_t0 + tch*512 + t, kt*128 + p]   (xT half-tiles)
    # w_d[og, p, kt*128 + o] = W[og*128 + o, kt*128 + p]          (WT per out-group)
    # wg_d[i, g*128 + o] = Wg[g, o, i]                            (WgT)
    # b_d[i, g] = b[g*128 + i];  bg_d[o, g] = bg[g, o]
    x_d = nc.dram_tensor("x", [KT, NCH, 128, NMOV], bf16, kind="ExternalInput")
    w_d = nc.dram_tensor("w", [G, 128, D], bf16, kind="ExternalInput")
    wg_d = nc.dram_tensor("wg", [128, G * IG], f32r, kind="ExternalInput")
    b_d = nc.dram_tensor("b", [128, G], f32, kind="ExternalInput")
    bg_d = nc.dram_tensor("bg", [128, G], f32, kind="ExternalInput")
    # o_d[og, o, t] = out[core_t0 + t, og*128 + o]                (outT)
    o_d = nc.dram_tensor("o", [G, 128, TPC], f32, kind="ExternalOutput")

    with tile.TileContext(nc) as tc:
        with (
            tc.tile_pool(name="xp", bufs=KT * NCH) as xp,
            tc.tile_pool(name="wp", bufs=6) as wp,
            tc.tile_pool(name="cp", bufs=1) as cp,
            tc.tile_pool(name="yp", bufs=18) as yp,
            tc.tile_pool(name="op", bufs=6) as op,
            tc.tile_pool(name="ps1", bufs=4, space=bass.MemorySpace.PSUM) as ps1,
            tc.tile_pool(name="ps2", bufs=4, space=bass.MemorySpace.PSUM) as ps2,
        ):
            w_tiles = {}

            def load_w(key):
                t = wp.tile([128, D], bf16, tag="w")
                nc.sync.dma_start(t[:], w_d[key[1]])
                w_tiles[key] = t

            # The first ~35us is DMA-bandwidth-bound, so queue order here IS
            # the schedule. The first RAMP groups run INTERLEAVED (kt-major
            # across RAMP psum banks) so each arriving x tile feeds RAMP
            # matmuls and the PE stays busy through the whole x wave; their W
            # tiles are delivered as just-in-time column chunks between the x
            # tiles they gate.
            RAMP = 4
            WCHUNK = 8            # kt-slices per ramp W chunk DMA
            b_sb = cp.tile([128, G], f32)
            nc.sync.dma_start(b_sb[:], b_d[:])
            ramp_w = []
            for og in range(RAMP):
                t = wp.tile([128, D], bf16, tag="w")
                ramp_w.append(t)
                w_tiles[(0, og)] = t
            x_sb = [[None] * NCH for _ in range(KT)]
            wg_sb = cp.tile([128, G * IG], f32r)
            bg_sb = cp.tile([128, G], f32)
            for c in range(KT // WCHUNK):
                lo, hi = c * WCHUNK * 128, (c + 1) * WCHUNK * 128
                for og in range(RAMP):
                    nc.sync.dma_start(
                        ramp_w[og][:, lo:hi], w_d[og][:, lo:hi]
                    )
                for kt in range(c * WCHUNK, (c + 1) * WCHUNK):
                    t = xp.tile([128, NMOV], bf16, tag="x")
                    nc.sync.dma_start(t[:], x_d[kt, 0])
                    x_sb[kt][0] = t
            load_w((0, RAMP))
            load_w((0, RAMP + 1))
            load_w((0, RAMP + 2))
            nc.sync.dma_start(wg_sb[:], wg_d[:])
            nc.sync.dma_start(bg_sb[:], bg_d[:])

            pending_q = deque()
            FLUSH_LAG = 6

            def flush_stage2(p):
                y_sb, og2, tch2 = p
                acc2 = ps2.tile([128, NMOV], f32, tag="acc2")
                nc.tensor.matmul(
                    acc2[:],
                    wg_sb[:, og2 * IG:(og2 + 1) * IG],
                    y_sb[:],
                    start=True,
                    stop=True,
                )
                o_sb = op.tile([128, NMOV], f32, tag="o")
                nc.scalar.activation(
                    o_sb[:], acc2[:], ACT_ID, bias=bg_sb[:, og2:og2 + 1]
                )
                nc.sync.dma_start(
                    o_d[og2][:, tch2 * NMOV:(tch2 + 1) * NMOV], o_sb[:]
                )

            # Interleaved ramp: RAMP accumulation groups advance together,
            # kt-major, one psum bank each, paced by the x-tile arrivals.
            accs = []
            for _r in range(RAMP):
                acc_r = ps1.tile([128, NMOV], f32, tag="acc")
                accs.append(acc_r)
            for kt in range(KT):
                for og in range(RAMP):
                    nc.tensor.matmul(
                        accs[og][:],
                        ramp_w[og][:, kt * 128:(kt + 1) * 128],
                        x_sb[kt][0][:],
                        start=(kt == 0),
                        stop=(kt == KT - 1),
                    )
            for og in range(RAMP):
                y_sb = yp.tile([128, NMOV], f32r, tag="y")
                nc.scalar.activation(
                    y_sb[:], accs[og][:], ACT_ID, bias=b_sb[:, og:og + 1]
                )
                pending_q.append((y_sb, og, 0))

            # tch outer: the whole first token-chunk pass (32 groups,
            # ~220us of matmul) runs before any tch=1 tile is needed, so the
            # second x wave has enormous DMA slack. W streams twice; at bf16
            # that is still far below the per-core HBM budget.
            passes = [(tch, og) for tch in range(NCH) for og in range(G)]
            for idx in range(RAMP, len(passes)):
                tch, og = passes[idx]
                w_sb = w_tiles.pop((tch, og))
                if idx + 3 < len(passes):
                    load_w(passes[idx + 3])
                # Trickle the second x wave in behind the W prefetches: two
                # 256 KB half-tiles per group keeps the W stream (needed in
                # ~2 groups) ahead of the x tiles (needed in ~28 groups).
                if idx - RAMP < KT // 2:
                    for kt in (2 * (idx - RAMP), 2 * (idx - RAMP) + 1):
                        t = xp.tile([128, NMOV], bf16, tag="x")
                        nc.sync.dma_start(t[:], x_d[kt, 1])
                        x_sb[kt][1] = t
                acc = ps1.tile([128, NMOV], f32, tag="acc")
                for kt in range(KT):
                    nc.tensor.matmul(
                        acc[:],
                        w_sb[:, kt * 128:(kt + 1) * 128],
                        x_sb[kt][tch][:],
                        start=(kt == 0),
                        stop=(kt == KT - 1),
                    )
                # Emit earlier iterations' grouped-stage matmuls with a
                # lag: their ACT producers ran during previous groups (the PE
                # never waits on the scalar engine) and the lag defers the
                # first use of wg past the DMA-bound ramp window.
                if len(pending_q) >= FLUSH_LAG:
                    flush_stage2(pending_q.popleft())
                y_sb = yp.tile([128, NMOV], f32r, tag="y")
                nc.scalar.activation(
                    y_sb[:], acc[:], ACT_ID, bias=b_sb[:, og:og + 1]
                )
                pending_q.append((y_sb, og, tch))
            while pending_q:
                flush_stage2(pending_q.popleft())

    nc.compile()
    return nc


def _get_nc():
    if "nc" not in _CACHE:
        _CACHE["nc"] = _build()
    return _CACHE["nc"]


def _run(x, W, b, Wg, bg, trace=False, tmpdir=None):
    x = np.ascontiguousarray(x, dtype=np.float32)
    W = np.ascontiguousarray(W, dtype=np.float32)
    b = np.ascontiguousarray(b, dtype=np.float32)
    Wg = np.ascontiguousarray(Wg, dtype=np.float32)
    bg = np.ascontiguousarray(bg, dtype=np.float32)

    # Host-side layout prep (pure permutes + weight casts, no math).
    # x: [B,S,D] -> per-core xT half-tiles [KT, NCH, 128, NMOV]
    x_dev = np.ascontiguousarray(
        x.reshape(NCORES, NCH, NMOV, KT, 128).transpose(0, 3, 1, 4, 2)
        .astype(ml_dtypes.bfloat16)
    )
    # W: [D_out, D_in] -> [og, p(k_local), kt*128 + o], bf16
    w_dev = np.ascontiguousarray(
        W.reshape(G, 128, KT, 128).transpose(0, 3, 2, 1).reshape(G, 128, D)
        .astype(ml_dtypes.bfloat16)
    )
    wg_dev = np.ascontiguousarray(
        Wg.transpose(2, 0, 1).reshape(128, G * IG)
    )
    b_dev = np.ascontiguousarray(b.reshape(G, 128).T)
    bg_dev = np.ascontiguousarray(bg.T)

    in_maps = [
        {"x": x_dev[c], "w": w_dev, "wg": wg_dev, "b": b_dev, "bg": bg_dev}
        for c in range(NCORES)
    ]
    nc = _get_nc()
    res = bass_utils.run_bass_kernel_spmd(
        nc, in_maps, core_ids=list(range(NCORES)), trace=trace, tmpdir=tmpdir
    )
    _CACHE["last_result"] = res

    out_t = np.concatenate(
        [res.results[c]["o"].reshape(D, TPC) for c in range(NCORES)], axis=1
    )
    return np.ascontiguousarray(out_t.T).reshape(B, S, D)


def kernel(x, W, b, Wg, bg):
    return _run(x, W, b, Wg, bg, trace=False)



# revision 3
# speedup vs baseline: 1.2597x; 1.2597x over previous
"""Trainium2 Bass kernel for EnhanceLayerLinear.

Computes out = GroupedLinear(Linear(x)):
    y = x @ W.T + b                      [B,S,D]
    out[..., g, :] = y[..., g, :] @ Wg[g].T + bg[g]   (block-diagonal, G groups)

The two stages fold into ONE dense GEMM: because the grouped stage is a
block-diagonal linear applied to y, we have

    out = x @ W'.T + b'   with   W'[g*128:(g+1)*128, :] = Wg[g] @ W[g*128:(g+1)*128, :]
                                 b' = blockdiag(Wg) @ b + bg

The fold costs 32 small [128x128]@[128x4096] host matmuls (~1.5% of total
FLOPs) and removes the 64 serialized f32r grouped-stage PE slots (the PE is
the bottleneck engine at >93% busy) plus their un-hidable 2-pass fp32
LDWEIGHTS and the end-of-kernel flush chain.

Sharding: data-parallel over tokens (B*S = 8192 -> 1024 per core). Each core
runs the single GEMM stage locally; no collectives.

Stage 1 runs in bf16 (fp32 accumulate in psum). The MM stream is pure
streaming-bound: one 512-col matmul per 512 PE cycles; LDWEIGHTS (bf16,
FWL-able) hides under the previous matmul via the PE reorder window.

Layout trick: y is computed TRANSPOSED (features on partitions, tokens on the
free axis), so each psum tile is one out-group's slice. The host hands the
kernel pre-transposed views of x / W' and re-transposes the output.

Schedule: the first ~35us is DMA-paced, so queue order IS the schedule.
x tiles are [128 x 1024] (full per-core token range, 2KB DMA lines); the
first W' column chunk and the first x tile are queued first so the PE starts
~10us in. Groups 0-3 ramp kt-major-interleaved (8 accumulation groups = all
8 psum banks), paced by the x wave; after the ramp all of x is SBUF-resident
and the remaining 28 groups run og-outer with W' streamed exactly once.
"""

import ml_dtypes
import numpy as np

import concourse.bacc as bacc
import concourse.bass as bass
import concourse.tile as tile
from concourse import mybir
from concourse import bass_utils

f32 = mybir.dt.float32
bf16 = mybir.dt.bfloat16
ACT_ID = mybir.ActivationFunctionType.Identity

B, S, D = 4, 2048, 4096
T = B * S                 # 8192 tokens
G, IG = 32, 128           # groups x group size (4096 = 32*128)
NCORES = 8
TPC = T // NCORES         # 1024 tokens per core
KT = D // 128             # 32 contraction tiles
NMOV = 512                # moving free dim per matmul (= one psum bank of fp32)
NCH = TPC // NMOV         # 2 token chunks per core
RAMP = 4                  # out-groups interleaved during the DMA-paced ramp
WCHUNK = 1024             # ramp W' column-chunk width (2KB DMA lines)

_CACHE = {}


def _build():
    nc = bacc.Bacc("TRN2", target_bir_lowering=False, debug=False)
    # x_d[kt, p, t] = x[core_t0 + t, kt*128 + p]        (xT tiles, 2KB lines)
    # w_d[og, p, kt*128 + o] = W'[og*128 + o, kt*128 + p]  (W'T per out-group)
    # b_d[i, g] = b'[g*128 + i]
    x_d = nc.dram_tensor("x", [KT, 128, TPC], bf16, kind="ExternalInput")
    w_d = nc.dram_tensor("w", [G, 128, D], bf16, kind="ExternalInput")
    b_d = nc.dram_tensor("b", [128, G], f32, kind="ExternalInput")
    # o_d[og, o, t] = out[core_t0 + t, og*128 + o]      (outT)
    o_d = nc.dram_tensor("o", [G, 128, TPC], f32, kind="ExternalOutput")

    with tile.TileContext(nc) as tc:
        with (
            tc.tile_pool(name="xp", bufs=KT) as xp,
            tc.tile_pool(name="wp", bufs=5) as wp,
            tc.tile_pool(name="cp", bufs=1) as cp,
            tc.tile_pool(name="op", bufs=8) as op,
            tc.tile_pool(name="ps", bufs=8, space=bass.MemorySpace.PSUM) as ps,
        ):
            w_tiles = {}

            def load_w(og):
                t = wp.tile([128, D], bf16, tag="w")
                nc.sync.dma_start(t[:], w_d[og])
                w_tiles[og] = t

            def emit_out(acc, og, tch):
                o_sb = op.tile([128, NMOV], f32, tag="o")
                nc.scalar.activation(
                    o_sb[:], acc[:], ACT_ID, bias=b_sb[:, og:og + 1]
                )
                nc.sync.dma_start(
                    o_d[og][:, tch * NMOV:(tch + 1) * NMOV], o_sb[:]
                )

            # --- DMA queue head: the critical path to the first matmul.
            # W' chunk (og0, cols 0:1024) then x tile 0, then the rest of the
            # ramp chunk-0 block interleaved so each arriving x tile has its
            # ramp weights already in flight.
            ramp_w = []
            for og in range(RAMP):
                t = wp.tile([128, D], bf16, tag="w")
                ramp_w.append(t)
                w_tiles[og] = t
            x_sb = [None] * KT

            def load_x(kt):
                t = xp.tile([128, TPC], bf16, tag="x")
                nc.sync.dma_start(t[:], x_d[kt])
                x_sb[kt] = t

            b_sb = cp.tile([128, G], f32)
            nc.sync.dma_start(ramp_w[0][:, 0:WCHUNK], w_d[0][:, 0:WCHUNK])
            load_x(0)
            for og in range(1, RAMP):
                nc.sync.dma_start(
                    ramp_w[og][:, 0:WCHUNK], w_d[og][:, 0:WCHUNK]
                )
                load_x(og)
            nc.sync.dma_start(b_sb[:], b_d[:])
            for kt in range(RAMP, 8):
                load_x(kt)
            for c in range(1, KT * 128 // WCHUNK):
                lo, hi = c * WCHUNK, (c + 1) * WCHUNK
                for og in range(RAMP):
                    nc.sync.dma_start(ramp_w[og][:, lo:hi], w_d[og][:, lo:hi])
                for kt in range(c * 8, (c + 1) * 8):
                    load_x(kt)
            load_w(RAMP)
            load_w(RAMP + 1)
            load_w(RAMP + 2)

            # --- Ramp: og 0..3 x both token chunks = 8 accumulation groups
            # (all 8 psum banks), advancing kt-major, paced by x arrivals.
            accs = {}
            for og in range(RAMP):
                for t in range(NCH):
                    accs[(og, t)] = ps.tile(
                        [128, NMOV], f32, tag="acc", name="acc"
                    )
            for kt in range(KT):
                for og in range(RAMP):
                    for t in range(NCH):
                        nc.tensor.matmul(
                            accs[(og, t)][:],
                            ramp_w[og][:, kt * 128:(kt + 1) * 128],
                            x_sb[kt][:, t * NMOV:(t + 1) * NMOV],
                            start=(kt == 0),
                            stop=(kt == KT - 1),
                        )
            for og in range(RAMP):
                for t in range(NCH):
                    emit_out(accs.pop((og, t)), og, t)

            # --- Steady state: og-outer, W' streamed once, x resident.
            for og in range(RAMP, G):
                w_sb = w_tiles.pop(og)
                if og + 3 < G:
                    load_w(og + 3)
                for tch in range(NCH):
                    acc = ps.tile([128, NMOV], f32, tag="acc")
                    for kt in range(KT):
                        nc.tensor.matmul(
                            acc[:],
                            w_sb[:, kt * 128:(kt + 1) * 128],
                            x_sb[kt][:, tch * NMOV:(tch + 1) * NMOV],
                            start=(kt == 0),
                            stop=(kt == KT - 1),
                        )
                    emit_out(acc, og, tch)

    nc.compile()
    return nc


def _get_nc():
    if "nc" not in _CACHE:
        _CACHE["nc"] = _build()
    return _CACHE["nc"]


def _run(x, W, b, Wg, bg, trace=False, tmpdir=None):
    x = np.ascontiguousarray(x, dtype=np.float32)
    W = np.ascontiguousarray(W, dtype=np.float32)
    b = np.ascontiguousarray(b, dtype=np.float32)
    Wg = np.ascontiguousarray(Wg, dtype=np.float32)
    bg = np.ascontiguousarray(bg, dtype=np.float32)

    # Fold the block-diagonal grouped stage into the dense weights:
    # W'[g] = Wg[g] @ W[g], b' = blockdiag(Wg) @ b + bg.
    Wf = np.matmul(Wg, W.reshape(G, IG, D)).reshape(D, D)
    bf = (np.matmul(Wg, b.reshape(G, IG, 1)).reshape(G, IG) + bg).reshape(D)

    # Host-side layout prep (permutes + casts).
    # x: [B,S,D] -> per-core xT tiles [KT, 128, TPC]
    x_dev = np.ascontiguousarray(
        x.reshape(NCORES, TPC, KT, 128).transpose(0, 2, 3, 1)
        .astype(ml_dtypes.bfloat16)
    )
    # W': [D_out, D_in] -> [og, p(k_local), kt*128 + o], bf16
    w_dev = np.ascontiguousarray(
        Wf.reshape(G, 128, KT, 128).transpose(0, 3, 2, 1).reshape(G, 128, D)
        .astype(ml_dtypes.bfloat16)
    )
    b_dev = np.ascontiguousarray(bf.reshape(G, 128).T)

    in_maps = [
        {"x": x_dev[c], "w": w_dev, "b": b_dev}
        for c in range(NCORES)
    ]
    nc = _get_nc()
    res = bass_utils.run_bass_kernel_spmd(
        nc, in_maps, core_ids=list(range(NCORES)), trace=trace, tmpdir=tmpdir
    )
    _CACHE["last_result"] = res

    out_t = np.concatenate(
        [res.results[c]["o"].reshape(D, TPC) for c in range(NCORES)], axis=1
    )
    return np.ascontiguousarray(out_t.T).reshape(B, S, D)


def kernel(x, W, b, Wg, bg):
    return _run(x, W, b, Wg, bg, trace=False)


# revision 4
# speedup vs baseline: 1.4284x; 1.1340x over previous
"""Trainium2 Bass kernel for EnhanceLayerLinear.

Computes out = GroupedLinear(Linear(x)):
    y = x @ W.T + b                      [B,S,D]
    out[..., g, :] = y[..., g, :] @ Wg[g].T + bg[g]   (block-diagonal, G groups)

The two stages fold into ONE dense GEMM: because the grouped stage is a
block-diagonal linear applied to y, we have

    out = x @ W'.T + b'   with   W'[g*128:(g+1)*128, :] = Wg[g] @ W[g*128:(g+1)*128, :]
                                 b' = blockdiag(Wg) @ b + bg

The fold costs 32 small [128x128]@[128x4096] host matmuls (~1.5% of total
FLOPs) and removes the 64 serialized f32r grouped-stage PE slots (the PE is
the bottleneck engine at >93% busy) plus their un-hidable 2-pass fp32
LDWEIGHTS and the end-of-kernel flush chain.

Sharding: data-parallel over tokens (B*S = 8192 -> 1024 per core). Each core
runs the single GEMM stage locally; no collectives.

Mixed precision: the PE streams one moving column per cycle in bf16, but fp8
with perf_mode=DoubleRow packs two contraction rows per cell and streams two
k-tiles per column-cycle. A full-fp8 GEMM misses the 2e-2 error gate, but a
PARTIAL-K split passes: the last M_FP8*2 of the 32 k-tiles run as fp8e4m3
DoubleRow pairs, the rest in bf16 (host-simulated exactly: rel-err 1.46e-2
at M_FP8=4 vs the 2e-2 gate; bf16-only is 1.74e-3). This converts
64 passes x 8 bf16 matmuls (216ns each) into 64 x 4 DR matmuls (~241ns),
~49us/core off the PE roofline.

Scaling: e4m3 has min-normal 2^-6, so raw x (std 1) and W' (std 0.0045)
must be rescaled into range: x_fp8 = e4m3(2^5 x), w_fp8 = e4m3(2^9 W').
Their psum contribution is then 2^14 too large, and psum accumulation cannot
apply a per-part scale -- so the bf16-part weights are pre-scaled by 2^14 as
well (exact in bf16: pure exponent shift) and the single psum accumulator is
evacuated with activation(scale=2^-14, bias=b'), which computes
func(in*scale + bias) in fp32.

Layout trick: y is computed TRANSPOSED (features on partitions, tokens on the
free axis), so each psum tile is one out-group's slice. The host hands the
kernel pre-transposed views of x / W' and re-transposes the output. fp8
operands are pair-packed for DoubleRow: 3D APs [128, 2, cols] where dim1
selects the k-tile of the pair.

Schedule: the first ~30us is DMA-paced, so queue order IS the schedule.
x tiles are [128 x 1024] (full per-core token range, 2KB DMA lines); the
first W' column chunk and the first x tile are queued first so the PE starts
~10us in. Groups 0-3 ramp kt-major-interleaved (8 accumulation groups = all
8 psum banks), paced by the x wave; after the ramp all of x is SBUF-resident
and the remaining 28 groups run og-outer with W' streamed exactly once.
"""

import ml_dtypes
import numpy as np

import concourse.bacc as bacc
import concourse.bass as bass
import concourse.tile as tile
from concourse import mybir
from concourse import bass_utils

f32 = mybir.dt.float32
bf16 = mybir.dt.bfloat16
fp8e4 = mybir.dt.float8e4
ACT_ID = mybir.ActivationFunctionType.Identity
DR = mybir.MatmulPerfMode.DoubleRow

B, S, D = 4, 2048, 4096
T = B * S                 # 8192 tokens
G, IG = 32, 128           # groups x group size (4096 = 32*128)
NCORES = 8
TPC = T // NCORES         # 1024 tokens per core
KT = D // 128             # 32 contraction tiles
M_FP8 = 4                 # fp8 DoubleRow k-tile PAIRS per pass (8 k-tiles)
KTB = KT - 2 * M_FP8      # bf16 k-tiles (24)
KB = KTB * 128            # bf16 contraction width (3072)
NMOV = 512                # moving free dim per matmul (= one psum bank of fp32)
NCH = TPC // NMOV         # 2 token chunks per core
RAMP = 4                  # out-groups interleaved during the DMA-paced ramp
WCHUNK = 1024             # ramp W' column-chunk width (2KB DMA lines)
SX = 2.0 ** 5             # fp8 x scale
SW = 2.0 ** 9             # fp8 W' scale
SOUT = 1.0 / (SX * SW)    # psum evacuation scale (2^-14)

_CACHE = {}


def _build():
    nc = bacc.Bacc("TRN2", target_bir_lowering=False, debug=False)
    # x_d[kt, p, t] = x[core_t0 + t, kt*128 + p]          (xT tiles, 2KB lines)
    # x8_d[j, p, i, t] = e4m3(SX * x[core_t0 + t, (KTB + 2j + i)*128 + p])
    # w_d[og, p, kt*128 + o] = bf16(SX*SW * W'[og*128 + o, kt*128 + p])
    # w8_d[og, p, i, j*128 + o] = e4m3(SW * W'[og*128 + o, (KTB + 2j + i)*128 + p])
    # b_d[i, g] = b'[g*128 + i]
    x_d = nc.dram_tensor("x", [KTB, 128, TPC], bf16, kind="ExternalInput")
    x8_d = nc.dram_tensor("x8", [M_FP8, 128, 2, TPC], fp8e4, kind="ExternalInput")
    w_d = nc.dram_tensor("w", [G, 128, KB], bf16, kind="ExternalInput")
    w8_d = nc.dram_tensor(
        "w8", [G, 128, 2, M_FP8 * 128], fp8e4, kind="ExternalInput"
    )
    b_d = nc.dram_tensor("b", [128, G], f32, kind="ExternalInput")
    # o_d[og, o, t] = out[core_t0 + t, og*128 + o]        (outT)
    o_d = nc.dram_tensor("o", [G, 128, TPC], f32, kind="ExternalOutput")

    with tile.TileContext(nc) as tc:
        with (
            tc.tile_pool(name="xp", bufs=KTB) as xp,
            tc.tile_pool(name="x8p", bufs=M_FP8) as x8p,
            tc.tile_pool(name="wp", bufs=5) as wp,
            tc.tile_pool(name="w8p", bufs=5) as w8p,
            tc.tile_pool(name="cp", bufs=1) as cp,
            tc.tile_pool(name="op", bufs=8) as op,
            tc.tile_pool(name="ps", bufs=8, space=bass.MemorySpace.PSUM) as ps,
        ):
            w_tiles = {}
            w8_tiles = {}

            def load_w(og):
                t = wp.tile([128, KB], bf16, tag="w", name="w")
                nc.sync.dma_start(t[:], w_d[og])
                w_tiles[og] = t
                t8 = w8p.tile([128, 2, M_FP8 * 128], fp8e4, tag="w8", name="w8")
                nc.sync.dma_start(t8[:], w8_d[og])
                w8_tiles[og] = t8

            def chain(acc, w_sb, w8_sb, tch, first):
                tlo, thi = tch * NMOV, (tch + 1) * NMOV
                for kt in range(KTB):
                    nc.tensor.matmul(
                        acc[:],
                        w_sb[:, kt * 128:(kt + 1) * 128],
                        x_sb[kt][:, tlo:thi],
                        start=(kt == 0),
                        stop=False,
                    )
                for j in range(M_FP8):
                    nc.tensor.matmul(
                        acc[:],
                        w8_sb[:, :, j * 128:(j + 1) * 128],
                        x8_sb[j][:, :, tlo:thi],
                        start=False,
                        stop=(j == M_FP8 - 1),
                        perf_mode=DR,
                    )

            def emit_out(acc, og, tch):
                o_sb = op.tile([128, NMOV], f32, tag="o", name="o_sb")
                nc.scalar.activation(
                    o_sb[:], acc[:], ACT_ID, bias=b_sb[:, og:og + 1], scale=SOUT
                )
                nc.sync.dma_start(
                    o_d[og][:, tch * NMOV:(tch + 1) * NMOV], o_sb[:]
                )

            # --- DMA queue head: the critical path to the first matmul.
            ramp_w = []
            ramp_w8 = []
            for og in range(RAMP):
                t = wp.tile([128, KB], bf16, tag="w", name="w")
                ramp_w.append(t)
                w_tiles[og] = t
                t8 = w8p.tile([128, 2, M_FP8 * 128], fp8e4, tag="w8", name="w8")
                ramp_w8.append(t8)
                w8_tiles[og] = t8
            x_sb = [None] * KTB
            x8_sb = [None] * M_FP8

            def load_x(kt):
                t = xp.tile([128, TPC], bf16, tag="x", name="x_sb")
                nc.sync.dma_start(t[:], x_d[kt])
                x_sb[kt] = t

            b_sb = cp.tile([128, G], f32)
            nc.sync.dma_start(ramp_w[0][:, 0:WCHUNK], w_d[0][:, 0:WCHUNK])
            load_x(0)
            for og in range(1, RAMP):
                nc.sync.dma_start(
                    ramp_w[og][:, 0:WCHUNK], w_d[og][:, 0:WCHUNK]
                )
                load_x(og)
            nc.sync.dma_start(b_sb[:], b_d[:])
            for kt in range(RAMP, 8):
                load_x(kt)
            for c in range(1, KB // WCHUNK):
                lo, hi = c * WCHUNK, (c + 1) * WCHUNK
                for og in range(RAMP):
                    nc.sync.dma_start(ramp_w[og][:, lo:hi], w_d[og][:, lo:hi])
                for kt in range(c * 8, min((c + 1) * 8, KTB)):
                    load_x(kt)
            for j in range(M_FP8):
                t8 = x8p.tile([128, 2, TPC], fp8e4, tag="x8", name="x8_sb")
                nc.sync.dma_start(t8[:], x8_d[j])
                x8_sb[j] = t8
            for og in range(RAMP):
                nc.sync.dma_start(ramp_w8[og][:], w8_d[og])
            load_w(RAMP)
            load_w(RAMP + 1)
            load_w(RAMP + 2)

            # --- Ramp: og 0..3 x both token chunks = 8 accumulation groups
            # (all 8 psum banks), advancing kt-major, paced by x arrivals.
            accs = {}
            for og in range(RAMP):
                for t in range(NCH):
                    accs[(og, t)] = ps.tile(
                        [128, NMOV], f32, tag="acc", name="acc"
                    )
            for kt in range(KTB):
                for og in range(RAMP):
                    for t in range(NCH):
                        nc.tensor.matmul(
                            accs[(og, t)][:],
                            ramp_w[og][:, kt * 128:(kt + 1) * 128],
                            x_sb[kt][:, t * NMOV:(t + 1) * NMOV],
                            start=(kt == 0),
                            stop=False,
                        )
            for j in range(M_FP8):
                for og in range(RAMP):
                    for t in range(NCH):
                        nc.tensor.matmul(
                            accs[(og, t)][:],
                            ramp_w8[og][:, :, j * 128:(j + 1) * 128],
                            x8_sb[j][:, :, t * NMOV:(t + 1) * NMOV],
                            start=False,
                            stop=(j == M_FP8 - 1),
                            perf_mode=DR,
                        )
            for og in range(RAMP):
                for t in range(NCH):
                    emit_out(accs.pop((og, t)), og, t)

            # --- Steady state: og-outer, W' streamed once, x resident.
            for og in range(RAMP, G):
                w_sb = w_tiles.pop(og)
                w8_sb = w8_tiles.pop(og)
                if og + 3 < G:
                    load_w(og + 3)
                for tch in range(NCH):
                    acc = ps.tile([128, NMOV], f32, tag="acc", name="acc")
                    chain(acc, w_sb, w8_sb, tch, og == RAMP and tch == 0)
                    emit_out(acc, og, tch)

    nc.compile()
    return nc


def _get_nc():
    if "nc" not in _CACHE:
        _CACHE["nc"] = _build()
    return _CACHE["nc"]


def _prep_inputs(x, W, b, Wg, bg):
    x = np.ascontiguousarray(x, dtype=np.float32)
    W = np.ascontiguousarray(W, dtype=np.float32)
    b = np.ascontiguousarray(b, dtype=np.float32)
    Wg = np.ascontiguousarray(Wg, dtype=np.float32)
    bg = np.ascontiguousarray(bg, dtype=np.float32)

    # Fold the block-diagonal grouped stage into the dense weights:
    # W'[g] = Wg[g] @ W[g], b' = blockdiag(Wg) @ b + bg.
    Wf = np.matmul(Wg, W.reshape(G, IG, D)).reshape(D, D)
    bf = (np.matmul(Wg, b.reshape(G, IG, 1)).reshape(G, IG) + bg).reshape(D)

    # x: [B,S,D] -> per-core xT tiles; bf16 part [KTB,128,TPC], fp8 pairs
    # [M_FP8,128,2,TPC] (DoubleRow pair-packed, dim "2" = k-tile of the pair).
    xr = x.reshape(NCORES, TPC, KT, 128)
    x_dev = np.ascontiguousarray(
        xr[:, :, :KTB, :].transpose(0, 2, 3, 1).astype(ml_dtypes.bfloat16)
    )
    x8_dev = np.ascontiguousarray(
        (SX * xr[:, :, KTB:, :])
        .reshape(NCORES, TPC, M_FP8, 2, 128)
        .transpose(0, 2, 4, 3, 1)
        .astype(ml_dtypes.float8_e4m3)
    )
    # W': [D_out, D_in] -> [og, p(k_local), kt*128 + o]; bf16 part pre-scaled
    # by SX*SW (exact exponent shift), fp8 pairs [og, p, i, j*128+o].
    wr = Wf.reshape(G, 128, KT, 128)
    w_dev = np.ascontiguousarray(
        (SX * SW * wr[:, :, :KTB, :])
        .transpose(0, 3, 2, 1)
        .reshape(G, 128, KB)
        .astype(ml_dtypes.bfloat16)
    )
    w8_dev = np.ascontiguousarray(
        (SW * wr[:, :, KTB:, :])
        .reshape(G, 128, M_FP8, 2, 128)
        .transpose(0, 4, 3, 2, 1)
        .reshape(G, 128, 2, M_FP8 * 128)
        .astype(ml_dtypes.float8_e4m3)
    )
    b_dev = np.ascontiguousarray(bf.reshape(G, 128).T.astype(np.float32))

    return [
        {
            "x": x_dev[c],
            "x8": x8_dev[c],
            "w": w_dev,
            "w8": w8_dev,
            "b": b_dev,
        }
        for c in range(NCORES)
    ]


def _run(x, W, b, Wg, bg, trace=False, tmpdir=None):
    in_maps = _prep_inputs(x, W, b, Wg, bg)
    nc = _get_nc()
    res = bass_utils.run_bass_kernel_spmd(
        nc, in_maps, core_ids=list(range(NCORES)), trace=trace, tmpdir=tmpdir
    )
    _CACHE["last_result"] = res

    out_t = np.concatenate(
        [res.results[c]["o"].reshape(D, TPC) for c in range(NCORES)], axis=1
    )
    return np.ascontiguousarray(out_t.T).reshape(B, S, D)


def kernel(x, W, b, Wg, bg):
    return _run(x, W, b, Wg, bg, trace=False)


# revision 6
# speedup vs baseline: 1.4709x; 1.0297x over previous
"""Trainium2 Bass kernel for EnhanceLayerLinear.

Computes out = GroupedLinear(Linear(x)):
    y = x @ W.T + b                      [B,S,D]
    out[..., g, :] = y[..., g, :] @ Wg[g].T + bg[g]   (block-diagonal, G groups)

The two stages fold into ONE dense GEMM: because the grouped stage is a
block-diagonal linear applied to y, we have

    out = x @ W'.T + b'   with   W'[g*128:(g+1)*128, :] = Wg[g] @ W[g*128:(g+1)*128, :]
                                 b' = blockdiag(Wg) @ b + bg

The fold costs 32 small [128x128]@[128x4096] host matmuls (~1.5% of total
FLOPs) and removes the 64 serialized f32r grouped-stage PE slots (the PE is
the bottleneck engine at >93% busy) plus their un-hidable 2-pass fp32
LDWEIGHTS and the end-of-kernel flush chain.

Sharding: data-parallel over tokens (B*S = 8192 -> 1024 per core). Each core
runs the single GEMM stage locally; no collectives.

Mixed precision: the PE streams one moving column per cycle in bf16, but fp8
with perf_mode=DoubleRow packs two contraction rows per cell and streams two
k-tiles per column-cycle. A full-fp8 GEMM misses the 2e-2 error gate, but a
PARTIAL-K split passes: the last M_FP8*2 of the 32 k-tiles run as fp8e4m3
DoubleRow pairs, the rest in bf16 (host-simulated exactly: rel-err 1.46e-2
at M_FP8=4 vs the 2e-2 gate; bf16-only is 1.74e-3). This converts
64 passes x 8 bf16 matmuls (216ns each) into 64 x 4 DR matmuls (~241ns),
~49us/core off the PE roofline.

Scaling: e4m3 has min-normal 2^-6, so raw x (std 1) and W' (std 0.0045)
must be rescaled into range: x_fp8 = e4m3(2^5 x), w_fp8 = e4m3(2^9 W').
Their psum contribution is then 2^14 too large, and psum accumulation cannot
apply a per-part scale -- so the bf16-part weights are pre-scaled by 2^14 as
well (exact in bf16: pure exponent shift) and the single psum accumulator is
evacuated with activation(scale=2^-14, bias=b'), which computes
func(in*scale + bias) in fp32.

Layout trick: y is computed TRANSPOSED (features on partitions, tokens on the
free axis), so each psum tile is one out-group's slice. The host hands the
kernel pre-transposed views of x / W' and re-transposes the output. fp8
operands are pair-packed for DoubleRow: 3D APs [128, 2, cols] where dim1
selects the k-tile of the pair.

Schedule: the first ~30us is DMA-paced, so queue order IS the schedule.
x tiles are [128 x 1024] (full per-core token range, 2KB DMA lines); the
first W' column chunk and the first x tile are queued first so the PE starts
~10us in. Groups 0-3 ramp kt-major-interleaved (8 accumulation groups = all
8 psum banks), paced by the x wave; after the ramp all of x is SBUF-resident
and the remaining 28 groups run og-outer with W' streamed exactly once.
"""

import ml_dtypes
import numpy as np

import concourse.bacc as bacc
import concourse.bass as bass
import concourse.tile as tile
from concourse import mybir
from concourse import bass_utils

f32 = mybir.dt.float32
bf16 = mybir.dt.bfloat16
fp8e4 = mybir.dt.float8e4
ACT_ID = mybir.ActivationFunctionType.Identity
DR = mybir.MatmulPerfMode.DoubleRow

B, S, D = 4, 2048, 4096
T = B * S                 # 8192 tokens
G, IG = 32, 128           # groups x group size (4096 = 32*128)
NCORES = 8
TPC = T // NCORES         # 1024 tokens per core
KT = D // 128             # 32 contraction tiles
M_FP8 = 5                 # fp8 DoubleRow k-tile PAIRS per pass (10 k-tiles)
KTB = KT - 2 * M_FP8      # bf16 k-tiles (24)
KB = KTB * 128            # bf16 contraction width (3072)
NMOV = 512                # moving free dim per matmul (= one psum bank of fp32)
NCH = TPC // NMOV         # 2 token chunks per core
RAMP = 4                  # out-groups interleaved during the DMA-paced ramp
WCHUNK = 1024             # ramp W' column-chunk width (2KB DMA lines)
SX = 2.0 ** 5             # fp8 x scale
SW = 2.0 ** 9             # fp8 W' scale
SOUT = 1.0 / (SX * SW)    # psum evacuation scale (2^-14)

_CACHE = {}


def _build():
    nc = bacc.Bacc("TRN2", target_bir_lowering=False, debug=False)
    # x_d[kt, p, t] = x[core_t0 + t, kt*128 + p]          (xT tiles, 2KB lines)
    # x8_d[j, p, i, t] = e4m3(SX * x[core_t0 + t, (KTB + 2j + i)*128 + p])
    # w_d[og, p, kt*128 + o] = bf16(SX*SW * W'[og*128 + o, kt*128 + p])
    # w8_d[og, p, i, j*128 + o] = e4m3(SW * W'[og*128 + o, (KTB + 2j + i)*128 + p])
    # b_d[i, g] = b'[g*128 + i]
    x_d = nc.dram_tensor("x", [KTB, 128, TPC], bf16, kind="ExternalInput")
    x8_d = nc.dram_tensor("x8", [M_FP8, 128, 2, TPC], fp8e4, kind="ExternalInput")
    w_d = nc.dram_tensor("w", [G, 128, KB], bf16, kind="ExternalInput")
    w8_d = nc.dram_tensor(
        "w8", [G, 128, 2, M_FP8 * 128], fp8e4, kind="ExternalInput"
    )
    b_d = nc.dram_tensor("b", [128, G], f32, kind="ExternalInput")
    # o_d[og, o, t] = out[core_t0 + t, og*128 + o]        (outT)
    o_d = nc.dram_tensor("o", [G, 128, TPC], f32, kind="ExternalOutput")

    with tile.TileContext(nc) as tc:
        with (
            tc.tile_pool(name="xp", bufs=KTB) as xp,
            tc.tile_pool(name="x8p", bufs=M_FP8) as x8p,
            tc.tile_pool(name="wp", bufs=5) as wp,
            tc.tile_pool(name="w8p", bufs=5) as w8p,
            tc.tile_pool(name="cp", bufs=1) as cp,
            tc.tile_pool(name="op", bufs=8) as op,
            tc.tile_pool(name="ps", bufs=8, space=bass.MemorySpace.PSUM) as ps,
        ):
            w_tiles = {}
            w8_tiles = {}

            def load_w(og):
                t = wp.tile([128, KB], bf16, tag="w", name="w")
                nc.sync.dma_start(t[:], w_d[og])
                w_tiles[og] = t
                t8 = w8p.tile([128, 2, M_FP8 * 128], fp8e4, tag="w8", name="w8")
                nc.sync.dma_start(t8[:], w8_d[og])
                w8_tiles[og] = t8

            def chain(acc, w_sb, w8_sb, tch, first):
                tlo, thi = tch * NMOV, (tch + 1) * NMOV
                for kt in range(KTB):
                    nc.tensor.matmul(
                        acc[:],
                        w_sb[:, kt * 128:(kt + 1) * 128],
                        x_sb[kt][:, tlo:thi],
                        start=(kt == 0),
                        stop=False,
                    )
                for j in range(M_FP8):
                    nc.tensor.matmul(
                        acc[:],
                        w8_sb[:, :, j * 128:(j + 1) * 128],
                        x8_sb[j][:, :, tlo:thi],
                        start=False,
                        stop=(j == M_FP8 - 1),
                        perf_mode=DR,
                    )

            def emit_out(acc, og, tch):
                o_sb = op.tile([128, NMOV], f32, tag="o", name="o_sb")
                nc.scalar.activation(
                    o_sb[:], acc[:], ACT_ID, bias=b_sb[:, og:og + 1], scale=SOUT
                )
                nc.sync.dma_start(
                    o_d[og][:, tch * NMOV:(tch + 1) * NMOV], o_sb[:]
                )

            # --- DMA queue head: the critical path to the first matmul.
            ramp_w = []
            ramp_w8 = []
            for og in range(RAMP):
                t = wp.tile([128, KB], bf16, tag="w", name="w")
                ramp_w.append(t)
                w_tiles[og] = t
                t8 = w8p.tile([128, 2, M_FP8 * 128], fp8e4, tag="w8", name="w8")
                ramp_w8.append(t8)
                w8_tiles[og] = t8
            x_sb = [None] * KTB
            x8_sb = [None] * M_FP8

            def load_x(kt):
                t = xp.tile([128, TPC], bf16, tag="x", name="x_sb")
                nc.sync.dma_start(t[:], x_d[kt])
                x_sb[kt] = t

            b_sb = cp.tile([128, G], f32)
            nc.sync.dma_start(ramp_w[0][:, 0:WCHUNK], w_d[0][:, 0:WCHUNK])
            load_x(0)
            for og in range(1, RAMP):
                nc.sync.dma_start(
                    ramp_w[og][:, 0:WCHUNK], w_d[og][:, 0:WCHUNK]
                )
                load_x(og)
            nc.sync.dma_start(b_sb[:], b_d[:])
            for kt in range(RAMP, 8):
                load_x(kt)
            for c in range(1, (KB + WCHUNK - 1) // WCHUNK):
                lo, hi = c * WCHUNK, min((c + 1) * WCHUNK, KB)
                for og in range(RAMP):
                    nc.sync.dma_start(ramp_w[og][:, lo:hi], w_d[og][:, lo:hi])
                for kt in range(c * 8, min((c + 1) * 8, KTB)):
                    load_x(kt)
            for j in range(M_FP8):
                t8 = x8p.tile([128, 2, TPC], fp8e4, tag="x8", name="x8_sb")
                nc.sync.dma_start(t8[:], x8_d[j])
                x8_sb[j] = t8
            for og in range(RAMP):
                nc.sync.dma_start(ramp_w8[og][:], w8_d[og])
            load_w(RAMP)
            load_w(RAMP + 1)
            load_w(RAMP + 2)

            # --- Ramp: og 0..3 x both token chunks = 8 accumulation groups
            # (all 8 psum banks), advancing kt-major, paced by x arrivals.
            accs = {}
            for og in range(RAMP):
                for t in range(NCH):
                    accs[(og, t)] = ps.tile(
                        [128, NMOV], f32, tag="acc", name="acc"
                    )
            for kt in range(KTB):
                for og in range(RAMP):
                    for t in range(NCH):
                        nc.tensor.matmul(
                            accs[(og, t)][:],
                            ramp_w[og][:, kt * 128:(kt + 1) * 128],
                            x_sb[kt][:, t * NMOV:(t + 1) * NMOV],
                            start=(kt == 0),
                            stop=False,
                        )
            for j in range(M_FP8):
                for og in range(RAMP):
                    for t in range(NCH):
                        nc.tensor.matmul(
                            accs[(og, t)][:],
                            ramp_w8[og][:, :, j * 128:(j + 1) * 128],
                            x8_sb[j][:, :, t * NMOV:(t + 1) * NMOV],
                            start=False,
                            stop=(j == M_FP8 - 1),
                            perf_mode=DR,
                        )
            for og in range(RAMP):
                for t in range(NCH):
                    emit_out(accs.pop((og, t)), og, t)

            # --- Steady state: og-outer, W' streamed once, x resident.
            for og in range(RAMP, G):
                w_sb = w_tiles.pop(og)
                w8_sb = w8_tiles.pop(og)
                if og + 3 < G:
                    load_w(og + 3)
                for tch in range(NCH):
                    acc = ps.tile([128, NMOV], f32, tag="acc", name="acc")
                    chain(acc, w_sb, w8_sb, tch, og == RAMP and tch == 0)
                    emit_out(acc, og, tch)

    nc.compile()
    return nc


def _get_nc():
    if "nc" not in _CACHE:
        _CACHE["nc"] = _build()
    return _CACHE["nc"]


def _prep_inputs(x, W, b, Wg, bg):
    x = np.ascontiguousarray(x, dtype=np.float32)
    W = np.ascontiguousarray(W, dtype=np.float32)
    b = np.ascontiguousarray(b, dtype=np.float32)
    Wg = np.ascontiguousarray(Wg, dtype=np.float32)
    bg = np.ascontiguousarray(bg, dtype=np.float32)

    # Fold the block-diagonal grouped stage into the dense weights:
    # W'[g] = Wg[g] @ W[g], b' = blockdiag(Wg) @ b + bg.
    Wf = np.matmul(Wg, W.reshape(G, IG, D)).reshape(D, D)
    bf = (np.matmul(Wg, b.reshape(G, IG, 1)).reshape(G, IG) + bg).reshape(D)

    # x: [B,S,D] -> per-core xT tiles; bf16 part [KTB,128,TPC], fp8 pairs
    # [M_FP8,128,2,TPC] (DoubleRow pair-packed, dim "2" = k-tile of the pair).
    xr = x.reshape(NCORES, TPC, KT, 128)
    x_dev = np.ascontiguousarray(
        xr[:, :, :KTB, :].transpose(0, 2, 3, 1).astype(ml_dtypes.bfloat16)
    )
    x8_dev = np.ascontiguousarray(
        (SX * xr[:, :, KTB:, :])
        .reshape(NCORES, TPC, M_FP8, 2, 128)
        .transpose(0, 2, 4, 3, 1)
        .astype(ml_dtypes.float8_e4m3)
    )
    # W': [D_out, D_in] -> [og, p(k_local), kt*128 + o]; bf16 part pre-scaled
    # by SX*SW (exact exponent shift), fp8 pairs [og, p, i, j*128+o].
    wr = Wf.reshape(G, 128, KT, 128)
    w_dev = np.ascontiguousarray(
        (SX * SW * wr[:, :, :KTB, :])
        .transpose(0, 3, 2, 1)
        .reshape(G, 128, KB)
        .astype(ml_dtypes.bfloat16)
    )
    w8_dev = np.ascontiguousarray(
        (SW * wr[:, :, KTB:, :])
        .reshape(G, 128, M_FP8, 2, 128)
        .transpose(0, 4, 3, 2, 1)
        .reshape(G, 128, 2, M_FP8 * 128)
        .astype(ml_dtypes.float8_e4m3)
    )
    b_dev = np.ascontiguousarray(bf.reshape(G, 128).T.astype(np.float32))

    return [
        {
            "x": x_dev[c],
            "x8": x8_dev[c],
            "w": w_dev,
            "w8": w8_dev,
            "b": b_dev,
        }
        for c in range(NCORES)
    ]


def _run(x, W, b, Wg, bg, trace=False, tmpdir=None):
    in_maps = _prep_inputs(x, W, b, Wg, bg)
    nc = _get_nc()
    res = bass_utils.run_bass_kernel_spmd(
        nc, in_maps, core_ids=list(range(NCORES)), trace=trace, tmpdir=tmpdir
    )
    _CACHE["last_result"] = res

    out_t = np.concatenate(
        [res.results[c]["o"].reshape(D, TPC) for c in range(NCORES)], axis=1
    )
    return np.ascontiguousarray(out_t.T).reshape(B, S, D)


def kernel(x, W, b, Wg, bg):
    return _run(x, W, b, Wg, bg, trace=False)


# revision 11
# speedup vs baseline: 1.4727x; 1.0012x over previous
"""Trainium2 Bass kernel for EnhanceLayerLinear.

Computes out = GroupedLinear(Linear(x)):
    y = x @ W.T + b                      [B,S,D]
    out[..., g, :] = y[..., g, :] @ Wg[g].T + bg[g]   (block-diagonal, G groups)

The two stages fold into ONE dense GEMM: because the grouped stage is a
block-diagonal linear applied to y, we have

    out = x @ W'.T + b'   with   W'[g*128:(g+1)*128, :] = Wg[g] @ W[g*128:(g+1)*128, :]
                                 b' = blockdiag(Wg) @ b + bg

The fold costs 32 small [128x128]@[128x4096] host matmuls (~1.5% of total
FLOPs) and removes the 64 serialized f32r grouped-stage PE slots (the PE is
the bottleneck engine at >93% busy) plus their un-hidable 2-pass fp32
LDWEIGHTS and the end-of-kernel flush chain.

Sharding: data-parallel over tokens (B*S = 8192 -> 1024 per core). Each core
runs the single GEMM stage locally; no collectives.

Mixed precision: the PE streams one moving column per cycle in bf16, but fp8
with perf_mode=DoubleRow packs two contraction rows per cell and streams two
k-tiles per column-cycle. A full-fp8 GEMM misses the 2e-2 error gate, but a
PARTIAL-K split passes: the last M_FP8*2 of the 32 k-tiles run as fp8e4m3
DoubleRow pairs, the rest in bf16 (host-simulated exactly: rel-err 1.46e-2
at M_FP8=4 vs the 2e-2 gate; bf16-only is 1.74e-3). This converts
64 passes x 8 bf16 matmuls (216ns each) into 64 x 4 DR matmuls (~241ns),
~49us/core off the PE roofline.

Scaling: e4m3 has min-normal 2^-6, so raw x (std 1) and W' (std 0.0045)
must be rescaled into range: x_fp8 = e4m3(2^5 x), w_fp8 = e4m3(2^9 W').
Their psum contribution is then 2^14 too large, and psum accumulation cannot
apply a per-part scale -- so the bf16-part weights are pre-scaled by 2^14 as
well (exact in bf16: pure exponent shift) and the single psum accumulator is
evacuated with activation(scale=2^-14, bias=b'), which computes
func(in*scale + bias) in fp32.

Layout trick: y is computed TRANSPOSED (features on partitions, tokens on the
free axis), so each psum tile is one out-group's slice. The host hands the
kernel pre-transposed views of x / W' and re-transposes the output. fp8
operands are pair-packed for DoubleRow: 3D APs [128, 2, cols] where dim1
selects the k-tile of the pair.

Schedule: the first ~30us is DMA-paced, so queue order IS the schedule.
x tiles are [128 x 1024] (full per-core token range, 2KB DMA lines); the
first W' column chunk and the first x tile are queued first so the PE starts
~10us in. Groups 0-3 ramp kt-major-interleaved (8 accumulation groups = all
8 psum banks), paced by the x wave; after the ramp all of x is SBUF-resident
and the remaining 28 groups run og-outer with W' streamed exactly once.
"""

import ml_dtypes
import numpy as np

import concourse.bacc as bacc
import concourse.bass as bass
import concourse.tile as tile
from concourse import mybir
from concourse import bass_utils

f32 = mybir.dt.float32
bf16 = mybir.dt.bfloat16
fp8e4 = mybir.dt.float8e4
ACT_ID = mybir.ActivationFunctionType.Identity
DR = mybir.MatmulPerfMode.DoubleRow

B, S, D = 4, 2048, 4096
T = B * S                 # 8192 tokens
G, IG = 32, 128           # groups x group size (4096 = 32*128)
NCORES = 8
TPC = T // NCORES         # 1024 tokens per core
KT = D // 128             # 32 contraction tiles
M_FP8 = 5                 # fp8 DoubleRow k-tile PAIRS per pass (10 k-tiles)
KTB = KT - 2 * M_FP8      # bf16 k-tiles (24)
KB = KTB * 128            # bf16 contraction width (3072)
NMOV = 512                # moving free dim per matmul (= one psum bank of fp32)
NCH = TPC // NMOV         # 2 token chunks per core
RAMP = 4                  # out-groups interleaved during the DMA-paced ramp
WCHUNK = 1024             # ramp W' column-chunk width (2KB DMA lines)
SX = 2.0 ** 5             # fp8 x scale
SW = 2.0 ** 9             # fp8 W' scale
SOUT = 1.0 / (SX * SW)    # psum evacuation scale (2^-14)

_CACHE = {}


def _build():
    nc = bacc.Bacc("TRN2", target_bir_lowering=False, debug=False)
    # x_d[kt, p, t] = x[core_t0 + t, kt*128 + p]          (xT tiles, 2KB lines)
    # x8_d[j, p, i, t] = e4m3(SX * x[core_t0 + t, (KTB + 2j + i)*128 + p])
    # w_d[og, p, kt*128 + o] = bf16(SX*SW * W'[og*128 + o, kt*128 + p])
    # w8_d[og, p, i, j*128 + o] = e4m3(SW * W'[og*128 + o, (KTB + 2j + i)*128 + p])
    # b_d[i, g] = b'[g*128 + i]
    x_d = nc.dram_tensor("x", [KTB, 128, TPC], bf16, kind="ExternalInput")
    x8_d = nc.dram_tensor("x8", [M_FP8, 128, 2, TPC], fp8e4, kind="ExternalInput")
    w_d = nc.dram_tensor("w", [G, 128, KB], bf16, kind="ExternalInput")
    w8_d = nc.dram_tensor(
        "w8", [G, 128, 2, M_FP8 * 128], fp8e4, kind="ExternalInput"
    )
    b_d = nc.dram_tensor("b", [128, G], f32, kind="ExternalInput")
    # o_d[og, o, t] = out[core_t0 + t, og*128 + o]        (outT)
    o_d = nc.dram_tensor("o", [G, 128, TPC], f32, kind="ExternalOutput")

    with tile.TileContext(nc) as tc:
        with (
            tc.tile_pool(name="xp", bufs=KTB) as xp,
            tc.tile_pool(name="x8p", bufs=M_FP8) as x8p,
            tc.tile_pool(name="wp", bufs=5) as wp,
            tc.tile_pool(name="w8p", bufs=5) as w8p,
            tc.tile_pool(name="cp", bufs=1) as cp,
            tc.tile_pool(name="op", bufs=8) as op,
            tc.tile_pool(name="ps", bufs=8, space=bass.MemorySpace.PSUM) as ps,
        ):
            w_tiles = {}
            w8_tiles = {}

            def load_w(og):
                t = wp.tile([128, KB], bf16, tag="w", name="w")
                nc.sync.dma_start(t[:], w_d[og])
                w_tiles[og] = t
                t8 = w8p.tile([128, 2, M_FP8 * 128], fp8e4, tag="w8", name="w8")
                nc.sync.dma_start(t8[:], w8_d[og])
                w8_tiles[og] = t8

            def chain(acc, w_sb, w8_sb, tch):
                tlo, thi = tch * NMOV, (tch + 1) * NMOV
                for kt in range(KTB):
                    nc.tensor.matmul(
                        acc[:],
                        w_sb[:, kt * 128:(kt + 1) * 128],
                        x_sb[kt][:, tlo:thi],
                        start=(kt == 0),
                        stop=False,
                    )
                for j in range(M_FP8):
                    nc.tensor.matmul(
                        acc[:],
                        w8_sb[:, :, j * 128:(j + 1) * 128],
                        x8_sb[j][:, :, tlo:thi],
                        start=False,
                        stop=(j == M_FP8 - 1),
                        perf_mode=DR,
                    )

            def emit_out(acc, og, tch):
                o_sb = op.tile([128, NMOV], f32, tag="o", name="o_sb")
                nc.scalar.activation(
                    o_sb[:], acc[:], ACT_ID, bias=b_sb[:, og:og + 1], scale=SOUT
                )
                # Issue the store from the Scalar queue: program-order after
                # its ACT, and keeps the Sync queue free for weight streaming.
                nc.scalar.dma_start(
                    o_d[og][:, tch * NMOV:(tch + 1) * NMOV], o_sb[:]
                )

            # --- DMA queue head: the critical path to the first matmul.
            ramp_w = []
            ramp_w8 = []
            for og in range(RAMP):
                t = wp.tile([128, KB], bf16, tag="w", name="w")
                ramp_w.append(t)
                w_tiles[og] = t
                t8 = w8p.tile([128, 2, M_FP8 * 128], fp8e4, tag="w8", name="w8")
                ramp_w8.append(t8)
                w8_tiles[og] = t8
            x_sb = [None] * KTB
            x8_sb = [None] * M_FP8

            def load_x(kt):
                t = xp.tile([128, TPC], bf16, tag="x", name="x_sb")
                nc.gpsimd.dma_start(t[:], x_d[kt])
                x_sb[kt] = t

            # The x stream issues from the (otherwise idle) GpSimd queue and
            # the W' stream from Sync, halving the serialized ~0.7us-per-
            # trigger cost on the ramp critical path. The first pieces are
            # small (W' 256 cols, x 512 tokens) so the first matmul fires as
            # early as possible.
            b_sb = cp.tile([128, G], f32)
            x0 = xp.tile([128, TPC], bf16, tag="x", name="x_sb")
            x_sb[0] = x0
            nc.gpsimd.dma_start(x0[:, 0:NMOV], x_d[0][:, 0:NMOV])
            for og in range(RAMP):
                nc.sync.dma_start(ramp_w[og][:, 0:256], w_d[og][:, 0:256])
            nc.gpsimd.dma_start(x0[:, NMOV:TPC], x_d[0][:, NMOV:TPC])
            for kt in range(1, 8):
                load_x(kt)
            for og in range(RAMP):
                nc.sync.dma_start(
                    ramp_w[og][:, 256:WCHUNK], w_d[og][:, 256:WCHUNK]
                )
            nc.gpsimd.dma_start(b_sb[:], b_d[:])
            for c in range(1, (KB + WCHUNK - 1) // WCHUNK):
                lo, hi = c * WCHUNK, min((c + 1) * WCHUNK, KB)
                for og in range(RAMP):
                    nc.sync.dma_start(ramp_w[og][:, lo:hi], w_d[og][:, lo:hi])
                for kt in range(c * 8, min((c + 1) * 8, KTB)):
                    load_x(kt)
            for j in range(M_FP8):
                t8 = x8p.tile([128, 2, TPC], fp8e4, tag="x8", name="x8_sb")
                nc.gpsimd.dma_start(t8[:], x8_d[j])
                x8_sb[j] = t8
            for og in range(RAMP):
                nc.sync.dma_start(ramp_w8[og][:], w8_d[og])
            load_w(RAMP)
            load_w(RAMP + 1)
            load_w(RAMP + 2)

            # --- Ramp: og 0..3 x both token chunks = 8 accumulation groups
            # (all 8 psum banks), advancing kt-major, paced by x arrivals.
            accs = {}
            for og in range(RAMP):
                for t in range(NCH):
                    accs[(og, t)] = ps.tile(
                        [128, NMOV], f32, tag="acc", name="acc"
                    )
            for kt in range(KTB):
                for og in range(RAMP):
                    for t in range(NCH):
                        nc.tensor.matmul(
                            accs[(og, t)][:],
                            ramp_w[og][:, kt * 128:(kt + 1) * 128],
                            x_sb[kt][:, t * NMOV:(t + 1) * NMOV],
                            start=(kt == 0),
                            stop=False,
                        )
            for j in range(M_FP8):
                for og in range(RAMP):
                    for t in range(NCH):
                        nc.tensor.matmul(
                            accs[(og, t)][:],
                            ramp_w8[og][:, :, j * 128:(j + 1) * 128],
                            x8_sb[j][:, :, t * NMOV:(t + 1) * NMOV],
                            start=False,
                            stop=(j == M_FP8 - 1),
                            perf_mode=DR,
                        )
            for og in range(RAMP):
                for t in range(NCH):
                    emit_out(accs.pop((og, t)), og, t)

            # --- Steady state: og-outer, W' streamed once, x resident.
            for og in range(RAMP, G):
                w_sb = w_tiles.pop(og)
                w8_sb = w8_tiles.pop(og)
                if og + 3 < G:
                    load_w(og + 3)
                for tch in range(NCH):
                    acc = ps.tile([128, NMOV], f32, tag="acc", name="acc")
                    chain(acc, w_sb, w8_sb, tch)
                    emit_out(acc, og, tch)

    nc.compile()
    return nc


def _get_nc():
    if "nc" not in _CACHE:
        _CACHE["nc"] = _build()
    return _CACHE["nc"]


def _prep_inputs(x, W, b, Wg, bg):
    x = np.ascontiguousarray(x, dtype=np.float32)
    W = np.ascontiguousarray(W, dtype=np.float32)
    b = np.ascontiguousarray(b, dtype=np.float32)
    Wg = np.ascontiguousarray(Wg, dtype=np.float32)
    bg = np.ascontiguousarray(bg, dtype=np.float32)

    # Fold the block-diagonal grouped stage into the dense weights:
    # W'[g] = Wg[g] @ W[g], b' = blockdiag(Wg) @ b + bg.
    Wf = np.matmul(Wg, W.reshape(G, IG, D)).reshape(D, D)
    bf = (np.matmul(Wg, b.reshape(G, IG, 1)).reshape(G, IG) + bg).reshape(D)

    # x: [B,S,D] -> per-core xT tiles; bf16 part [KTB,128,TPC], fp8 pairs
    # [M_FP8,128,2,TPC] (DoubleRow pair-packed, dim "2" = k-tile of the pair).
    xr = x.reshape(NCORES, TPC, KT, 128)
    x_dev = np.ascontiguousarray(
        xr[:, :, :KTB, :].transpose(0, 2, 3, 1).astype(ml_dtypes.bfloat16)
    )
    x8_dev = np.ascontiguousarray(
        (SX * xr[:, :, KTB:, :])
        .reshape(NCORES, TPC, M_FP8, 2, 128)
        .transpose(0, 2, 4, 3, 1)
        .astype(ml_dtypes.float8_e4m3)
    )
    # W': [D_out, D_in] -> [og, p(k_local), kt*128 + o]; bf16 part pre-scaled
    # by SX*SW (exact exponent shift), fp8 pairs [og, p, i, j*128+o].
    wr = Wf.reshape(G, 128, KT, 128)
    w_dev = np.ascontiguousarray(
        (SX * SW * wr[:, :, :KTB, :])
        .transpose(0, 3, 2, 1)
        .reshape(G, 128, KB)
        .astype(ml_dtypes.bfloat16)
    )
    w8_dev = np.ascontiguousarray(
        (SW * wr[:, :, KTB:, :])
        .reshape(G, 128, M_FP8, 2, 128)
        .transpose(0, 4, 3, 2, 1)
        .reshape(G, 128, 2, M_FP8 * 128)
        .astype(ml_dtypes.float8_e4m3)
    )
    b_dev = np.ascontiguousarray(bf.reshape(G, 128).T.astype(np.float32))

    return [
        {
            "x": x_dev[c],
            "x8": x8_dev[c],
            "w": w_dev,
            "w8": w8_dev,
            "b": b_dev,
        }
        for c in range(NCORES)
    ]


def _run(x, W, b, Wg, bg, trace=False, tmpdir=None):
    in_maps = _prep_inputs(x, W, b, Wg, bg)
    nc = _get_nc()
    res = bass_utils.run_bass_kernel_spmd(
        nc, in_maps, core_ids=list(range(NCORES)), trace=trace, tmpdir=tmpdir
    )
    _CACHE["last_result"] = res

    out_t = np.concatenate(
        [res.results[c]["o"].reshape(D, TPC) for c in range(NCORES)], axis=1
    )
    return np.ascontiguousarray(out_t.T).reshape(B, S, D)


def kernel(x, W, b, Wg, bg):
    return _run(x, W, b, Wg, bg, trace=False)
